# revision 61
# baseline (speedup 1.0000x reference)
"""Trainium2 Bass kernel for nn_MemorizingTransformer (retrieval_knn).

Memorizing-transformer attention block: cosine-sim causal local attention with
per-query retrieved KNN memories, joint softmax over [memory | local], and
input/output projections.

Sharding: (b, h) across 8 cores — core c handles batch b=c//4 and heads
h0=2*(c%4), h0+1. Every core runs an identical NEFF (pure SPMD); only input
slices differ. The output projection is computed per-core on the core's two
head rows of w_out, giving partial sums that the host reduces.

16-bit streaming version: x/weights/mem_k are host-cast to fp16 and mem_v to
bf16, halving HBM traffic (the roofline term) and putting every matmul at
1 cycle/row. Everything downstream of exp (P_T, P_mem, v, stage, output
partials) is bf16, NOT fp16: the shift-free softmax emits values down to
e^-40, far below fp16's exponent range, and fp16 there distorts num/den
ratios (abs err ~0.5). PSUM accumulation stays f32.

Device algorithm per core:
  xT      : 4 x DMA-transpose (xbar) of fp16 x               (DMA)
  q,v     : x @ [wq0 wq1 wv] per 128-token block              (PE)
            per-4-block pipeline: sumsq(q) (DVE mul+reduce), sqrt (ACT),
            recip (DVE), q_s16 = q*rn_q fp16, qT via PE transpose — so the
            mem-score chain starts ~8us into the kernel.
  kT      : wk^T @ xT directly in transposed layout, trailing the q chunks.
            k is never normalized on-chip: 1/|k_j| is folded into the
            per-partition exp scale of the local-attention softmax.
            |k|^2 via kT*kT (DVE) + ones-matmul (PE).
  per head p, per 4-block bank qc4 (memk feeds local feeds memv, so all
  engines pipeline across banks):
    mem scores per 2-block pair (one DVE op covers both blocks):
      prod = mem_k * q_s16 (fp16 2x mode); reduce has no DVE 2x mode, so
      four halving adds (2x) shrink it 16x first; s_mem fp16
      P_mem = exp(scale*S_mem - scale) -> bf16                (ACT)
    local, jt-outer with 4-block column batching:
      S_T[128j, <=512q] = kT.T @ qT_all                       (PE)
      P_T = exp((scale/|k_j|)*S_T - scale) bf16, tril on diag (ACT, DVE)
      PV: psum_o[g] += P_T_g.T @ [v|1]                        (PE accum)
    mem values per g (PE, block-diagonal trick):
      pmT = 32x32 stream-transpose of P_mem                   (DVE)
      stT staged block-diagonally via gpsimd copies           (Pool)
      32 small matmuls give pm[65, 128q] per group; transposed-accumulated
      into psum_o so col 64 = total softmax denominator.
    combine: oh = psum_o[:, :64] * recip(psum_o[:, 64]) (ACT mul w/ scale);
             hoT = transpose(oh); head-1 PSUM->SBUF copies ride DVE (ACT is
             the tail pacer)
  partial_out[g] = hoT_g.T @ w_out[2 head rows], fp16 partials (PE)

Softmax needs no max-subtraction: scores are cosine sims in [-1,1] times
scale=exp(scale_param), so exp(scale*(s-1)) is bounded in (0, 1].

Engine-balance notes (TimelineSim): DMA ~109us busy (the floor: 16.8MB mem_k
+ 17MB mem_v + x/out/weights per core), DVE ~97us, ACT ~98us, PE ~48us.
"""

import os
import numpy as np
import ml_dtypes
_bf16np = ml_dtypes.bfloat16

HEADS = 8
D = 64
KNN = 32
B = 2
N = 2048
DIM = 512
P = 128
NB = N // P          # 16 query/key blocks
NCO = DIM // P       # 4 contraction chunks of the model dim
NCORES = 8
PHASE_MARKS = []
_MSTATE = {}


def _mark(nc, name):
    cur = nc.next_id()
    if _MSTATE.get("name") is not None:
        PHASE_MARKS.append((_MSTATE["name"], _MSTATE["id"], cur))
    _MSTATE["name"] = name
    _MSTATE["id"] = cur


def _build(use_mbias: bool):
    import concourse.bass as bass
    import concourse.mybir as mybir
    import concourse.tile as tile
    from concourse import bacc

    f32 = mybir.dt.float32
    f16 = mybir.dt.float16
    bf16 = mybir.dt.bfloat16
    AX = mybir.AxisListType
    ACTF = mybir.ActivationFunctionType

    nc = bacc.Bacc(None, target_bir_lowering=False, name="memxformer")
    PHASE_MARKS.clear()
    _MSTATE.clear()

    # ---- I/O ------------------------------------------------------------
    xb = nc.dram_tensor("xb", (N, DIM), f16, kind="ExternalInput")
    # wqv = [wq_h0 | wq_h1 | wv] columns (192), wk separately (64)
    wqv = nc.dram_tensor("wqv", (DIM, 3 * D), f16, kind="ExternalInput")
    wk = nc.dram_tensor("wk", (DIM, D), f16, kind="ExternalInput")
    wout2 = nc.dram_tensor("wout2", (2 * D, DIM), f16, kind="ExternalInput")
    # scales[:, 0:2] = exp(scale_param[h0 + p]); scales[:, 2:4] = -that
    scales = nc.dram_tensor("scales", (P, 4), f32, kind="ExternalInput")
    mk = nc.dram_tensor("mk", (2, NB, P, KNN, D), f16, kind="ExternalInput")
    mv = nc.dram_tensor("mv", (2, NB, P, KNN, D + 1), bf16, kind="ExternalInput")
    if use_mbias:
        mbias = nc.dram_tensor("mbias", (2, NB, P, KNN), f16,
                               kind="ExternalInput")
    out = nc.dram_tensor("out", (N, DIM), f16, kind="ExternalOutput")

    # constants baked into the NEFF: one u16 blob = [eye16 fp16 | tril bf16 |
    # ones fp16] so the ramp pays a single const DMA
    combo_np = np.zeros((P, 2 * P + 1), dtype=np.uint16)
    combo_np[:, 0:P] = np.eye(P, dtype=np.float16).view(np.uint16)
    combo_np[:, P:2 * P] = np.triu(  # keep j <= q
        np.ones((P, P), dtype=_bf16np)).view(np.uint16)
    combo_np[0:D, 2 * P] = np.ones((D,), dtype=np.float16).view(np.uint16)
    combo_d = nc.inline_tensor(combo_np, name="combo_c")
    eye65_d = nc.inline_tensor(np.eye(D + 1, dtype=np.float32), name="eye65_c")

    with tile.TileContext(nc) as tc:
        with (
            tc.tile_pool(name="singles", bufs=1) as singles,
            tc.tile_pool(name="mem", bufs=8) as memp,
            tc.tile_pool(name="mvp", bufs=6) as mvp,
            tc.tile_pool(name="prods", bufs=2) as prods,
            tc.tile_pool(name="small", bufs=6) as small,
            tc.tile_pool(name="pt", bufs=3) as ptp,
            tc.tile_pool(name="pms", bufs=3) as pms,
            tc.tile_pool(name="outp", bufs=2) as outp,
            tc.tile_pool(name="ppt", bufs=2, space="PSUM") as ppt,
            tc.tile_pool(name="pp512", bufs=2, space="PSUM") as pp512,
            tc.tile_pool(name="ppo", bufs=4, space="PSUM") as ppo,
        ):
            # ---- constants / weights ------------------------------------
            combo_sb = singles.tile([P, 2 * P + 1], mybir.dt.uint16,
                                    tag="combo")
            nc.sync.dma_start(combo_sb, combo_d[:, :])
            eye16_sb = combo_sb[:, 0:P].bitcast(f16)
            tril_sb = combo_sb[:, P:2 * P].bitcast(bf16)
            ones_sb = combo_sb[0:D, 2 * P:2 * P + 1].bitcast(f16)
            eye65_sb = singles.tile([D + 1, D + 1], f32, tag="eye65")
            nc.sync.dma_start(eye65_sb, eye65_d[:, :])
            sc_sb = singles.tile([P, 4], f32, tag="scales")
            nc.sync.dma_start(sc_sb, scales[:, :])
            wqv_sb = singles.tile([P, NCO, 3 * D], f16, tag="wqv")
            nc.sync.dma_start(wqv_sb,
                              wqv[:, :].rearrange("(co p) c -> p co c", p=P))
            wk_sb = singles.tile([P, NCO, D], f16, tag="wk")
            nc.sync.dma_start(wk_sb,
                              wk[:, :].rearrange("(co p) c -> p co c", p=P))
            wout_sb = singles.tile([P, DIM], f16, tag="wout")
            nc.sync.dma_start(wout_sb, wout2[:, :])

            _mark(nc, "setup")
            # ---- x transpose via DMA xbar: xT[p, co, n] = x[n, co*128+p] -
            xT = singles.tile([P, NCO, N], f16, tag="xT")
            for co in range(NCO):
                nc.sync.dma_start_transpose(xT[:, co, :],
                                            xb[:, co * P:(co + 1) * P])

            _mark(nc, "kT")
            # ---- kT = wk^T @ xT (transposed layout, unnormalized) --------
            kT = singles.tile([D, NB, P], f16, tag="kT")
            kT2 = singles.tile([D, NB, P], f16, tag="kT2")
            pss = ppo.tile([P, NB], f32, tag="po", name="pss")

            def kT_chunk(ch):
                nsl = slice(ch * DIM, (ch + 1) * DIM)
                pkv = pp512.tile([D, DIM], f32, tag="p512")
                for co in range(NCO):
                    nc.tensor.matmul(pkv, wk_sb[:, co, :], xT[:, co, nsl],
                                     start=(co == 0), stop=(co == NCO - 1))
                nc.scalar.copy(out=kT[:, 4 * ch:4 * ch + 4, :],
                               in_=pkv.rearrange("p (g q) -> p g q", q=P))
                nc.vector.tensor_mul(kT2[:, 4 * ch:4 * ch + 4, :],
                                     kT[:, 4 * ch:4 * ch + 4, :],
                                     kT[:, 4 * ch:4 * ch + 4, :])
                for j4 in range(4):
                    jt = 4 * ch + j4
                    nc.tensor.matmul(pss[:, jt:jt + 1], kT2[:, jt, :], ones_sb,
                                     start=True, stop=True)

            _mark(nc, "qv")
            # ---- q (2 heads) + v natural projections + q sumsq -----------
            # q_all/q_s16 layout: [P, p, g, D]. Normalization runs per
            # 4-block chunk so the memk DVE chain can start early; Copy and
            # Sqrt share an ACT table, so per-chunk sqrts cost no reloads.
            q_all = singles.tile([P, 2, NB, D], f16, tag="q_all")
            q2 = singles.tile([P, 2, NB, D], f16, tag="q2")
            v_aug = singles.tile([P, NB, D + 1], bf16, tag="vaug")
            nc.gpsimd.memset(v_aug[:, :, D:D + 1], 1.0)
            ssq = singles.tile([P, 2, NB], f32, tag="ssq")
            nrm_q = singles.tile([P, 2, NB], f32, tag="nrm_q")
            rn_q = singles.tile([P, 2, NB], f32, tag="rn_q")
            q_s16 = singles.tile([P, 2, NB, D], f16, tag="q_s16")
            qT_all = singles.tile([D, 2, NB, P], f16, tag="qT")
            for gc in range(0, NB, 4):
                for g in range(gc, gc + 4):
                    qsl = slice(g * P, (g + 1) * P)
                    pqv = ppt.tile([P, 3 * D], f32, tag="tps")
                    for co in range(NCO):
                        nc.tensor.matmul(pqv, xT[:, co, qsl], wqv_sb[:, co, :],
                                         start=(co == 0), stop=(co == NCO - 1))
                    nc.scalar.copy(out=q_all[:, :, g, :],
                                   in_=pqv[:, 0:2 * D].rearrange(
                                       "p (a b) -> p a b", a=2))
                    nc.scalar.copy(out=v_aug[:, g, 0:D],
                                   in_=pqv[:, 2 * D:3 * D])
                gsl = slice(gc, gc + 4)
                nc.vector.tensor_mul(q2[:, :, gsl, :], q_all[:, :, gsl, :],
                                     q_all[:, :, gsl, :])
                nc.vector.reduce_sum(
                    ssq[:, :, gsl].rearrange("p a g -> p a g ()"),
                    q2[:, :, gsl, :], axis=AX.X)
                nc.scalar.sqrt(nrm_q[:, :, gsl], ssq[:, :, gsl])
                nc.vector.reciprocal(rn_q[:, :, gsl], nrm_q[:, :, gsl])
                for p in range(2):
                    pqt = ppt.tile([D, 4, P], f16, tag="tps")
                    for i4 in range(4):
                        g = gc + i4
                        nc.vector.tensor_scalar_mul(q_s16[:, p, g, :],
                                                    q_all[:, p, g, :],
                                                    rn_q[:, p, g:g + 1])
                        nc.tensor.transpose(pqt[:, i4, :], q_s16[:, p, g, :],
                                            eye16_sb)
                    nc.scalar.copy(out=qT_all[:, p, gc:gc + 4, :], in_=pqt)
                # k chunks trail the q chunks so q_s16 (the memk gate) is
                # produced as early as possible
                kT_chunk(gc // 4)

            _mark(nc, "norm")
            # ---- k norms: fold 1/|k_j| into the local exp scale ----------
            nrm_k = singles.tile([P, NB], f32, tag="nrm_k")
            nc.scalar.sqrt(nrm_k, pss)
            rn_k = singles.tile([P, NB], f32, tag="rn_k")
            nc.vector.reciprocal(rn_k, nrm_k)
            # per-head local-attention exp scale: exp(sp)*rn_k per partition j
            sc2 = singles.tile([P, 2, NB], f32, tag="sc2")
            for p in range(2):
                nc.vector.tensor_scalar_mul(sc2[:, p, :], rn_k,
                                            sc_sb[:, p:p + 1])

            # ---- head-output accumulator --------------------------------
            hoT = singles.tile([P, NB, P], f16, tag="hoT")
            # block-diagonal P_mem staging (manual double buffer; the
            # off-diagonal zeros are written once and never touched again)
            stT2 = singles.tile([P, 2, 4, P], bf16, tag="stT2")
            nc.gpsimd.memset(stT2, 0.0)

            for p in range(2):
                sc_ap = sc_sb[:, p:p + 1]
                nb_ap = sc_sb[:, 2 + p:3 + p]

                _mark(nc, "memk")
                # --- memory attention scores, per 2-block pair ---
                p_mem_all = singles.tile([P, NB, KNN], bf16, tag=f"pmem{p}")

                def memk_pair(g2):
                    mk_t = memp.tile([P, 2, KNN, D], f16, tag="mk")
                    nc.sync.dma_start(
                        mk_t, mk[p, 2 * g2:2 * g2 + 2].rearrange(
                            "g p k d -> p g k d"))
                    s_mem2 = small.tile([P, 2, KNN], f16, tag="smem")
                    prod = prods.tile([P, 2, KNN, D], f16, tag="prod")
                    nc.vector.tensor_mul(
                        prod, mk_t,
                        q_s16[:, p, 2 * g2:2 * g2 + 2, None, :].to_broadcast(
                            (P, 2, KNN, D)))
                    # TensorReduce has no 2x fp16 mode on DVE; shrink the
                    # reduce with 2x-mode adds first.
                    nc.vector.tensor_add(prod[:, :, :, 0:D // 2],
                                         prod[:, :, :, 0:D // 2],
                                         prod[:, :, :, D // 2:D])
                    nc.vector.tensor_add(prod[:, :, :, 0:D // 4],
                                         prod[:, :, :, 0:D // 4],
                                         prod[:, :, :, D // 4:D // 2])
                    nc.vector.tensor_add(prod[:, :, :, 0:D // 8],
                                         prod[:, :, :, 0:D // 8],
                                         prod[:, :, :, D // 8:D // 4])
                    nc.vector.tensor_add(prod[:, :, :, 0:D // 16],
                                         prod[:, :, :, 0:D // 16],
                                         prod[:, :, :, D // 16:D // 8])
                    with nc.allow_low_precision(
                            reason="fp16 KNN scores; DVE accumulates "
                                   "internally at higher precision"):
                        nc.vector.reduce_sum(s_mem2, prod[:, :, :, 0:D // 16],
                                             axis=AX.X)
                    if use_mbias:
                        mb_t = small.tile([P, 2, KNN], f16, tag="mbias")
                        nc.sync.dma_start(
                            mb_t, mbias[p, 2 * g2:2 * g2 + 2].rearrange(
                                "g p k -> p g k"))
                        nc.vector.tensor_add(s_mem2, s_mem2, mb_t)
                    nc.scalar.activation(
                        out=p_mem_all[:, 2 * g2:2 * g2 + 2, :],
                        in_=s_mem2.rearrange("p a k -> p (a k)"),
                        func=ACTF.Exp, bias=nb_ap, scale=sc_ap)

                _mark(nc, "local")
                # --- local causal attention, jt-outer, 4-block columns ---
                psum_o = [ppo.tile([P, 4, D + 1], f32, tag="po", name=f"po{i}")
                          for i in range(4)]

                def local_tile(qc, jt):
                    g_lo = max(jt, 4 * qc)
                    g_hi = 4 * qc + 4
                    ng = g_hi - g_lo
                    st_ps = pp512.tile([P, 512], f32, tag="p512",
                                       name="st_ps")
                    nc.tensor.matmul(
                        st_ps[:, :ng * P], kT[:, jt, :],
                        qT_all[:, p, g_lo:g_lo + ng, :],
                        start=True, stop=True)
                    p_t = ptp.tile([P, 4, P], bf16, tag="pt", name="p_t")
                    nc.scalar.activation(
                        out=p_t[:, :ng, :],
                        in_=st_ps[:, :ng * P].rearrange("p (g q) -> p g q",
                                                        q=P),
                        func=ACTF.Exp, bias=nb_ap, scale=sc2[:, p, jt:jt + 1])
                    if g_lo <= jt < g_hi:
                        di = jt - g_lo
                        nc.vector.tensor_mul(p_t[:, di, :], p_t[:, di, :],
                                             tril_sb)
                    for gi in range(ng):
                        g = g_lo + gi
                        nc.tensor.matmul(
                            psum_o[qc][:, g - 4 * qc, :], p_t[:, gi, :],
                            v_aug[:, jt, :],
                            start=(jt == 0 and gi == 0), stop=False)

                _mark(nc, "memv")
                # qc-outer: each bank's memk scores land just ahead of its
                # local attention; the memory-value chain fires right after
                # (overlaps later banks)
                for qc4 in range(4):
                    memk_pair(2 * qc4)
                    memk_pair(2 * qc4 + 1)
                    for jt in range(4 * qc4 + 4):
                        local_tile(qc4, jt)
                    gc = 4 * qc4
                    stage4 = stT2[:, (gc // 4) % 2, :, :]
                    for gi in range(4):
                        g = gc + gi
                        pmT = small.tile([P, KNN], bf16, tag="pmT")
                        nc.vector.transpose(pmT, p_mem_all[:, g, :])
                        for k4 in range(4):
                            nc.gpsimd.tensor_copy(
                                out=stage4[32 * k4:32 * (k4 + 1), gi,
                                           32 * k4:32 * (k4 + 1)],
                                in_=pmT[32 * k4:32 * (k4 + 1), :])
                    pm_ps = ppt.tile([D + 1, 4, P], f32, tag="tps",
                                     name="pm_ps")
                    for g2 in range(2):
                        mv_t = mvp.tile([P, 2, KNN, D + 1], bf16, tag="mv")
                        nc.sync.dma_start(
                            mv_t, mv[p, gc + 2 * g2:gc + 2 * g2 + 2].rearrange(
                                "g p k d -> p g k d"))
                        for gj in range(2):
                            gi = 2 * g2 + gj
                            stT_v = stage4[:, gi, :].rearrange(
                                "p (ql gf) -> p gf ql", gf=KNN)
                            pm_v = pm_ps[:, gi, :].rearrange(
                                "p (ql gf) -> p gf ql", gf=KNN)
                            for g4 in range(KNN):
                                nc.tensor.matmul(pm_v[:, g4, :],
                                                 mv_t[:, gj, g4, :],
                                                 stT_v[:, g4, :],
                                                 start=True, stop=True)
                    pm_sb = pms.tile([D + 1, 4, P], f32, tag="pm")
                    if p == 0:
                        nc.scalar.copy(out=pm_sb, in_=pm_ps)
                    else:
                        nc.vector.tensor_copy(out=pm_sb, in_=pm_ps)
                    ohp = ppt.tile([D, 4, P], f16, tag="tps")
                    for gi in range(4):
                        g = gc + gi
                        qc, gq = g // 4, g % 4
                        nc.tensor.matmul(psum_o[qc][:, gq, :], pm_sb[:, gi, :],
                                         eye65_sb,
                                         is_transpose=True, start=False,
                                         stop=True)
                        rcp = small.tile([P, 1], f32, tag="rcp")
                        nc.vector.reciprocal(rcp, psum_o[qc][:, gq, D:D + 1])
                        oh = small.tile([P, D], f16, tag="oh")
                        nc.scalar.mul(oh, psum_o[qc][:, gq, 0:D], rcp)
                        nc.tensor.transpose(ohp[:, gi, :], oh, eye16_sb)
                    if p == 0:
                        nc.scalar.copy(
                            out=hoT[p * D:(p + 1) * D, gc:gc + 4, :], in_=ohp)
                    else:
                        nc.vector.tensor_copy(
                            out=hoT[p * D:(p + 1) * D, gc:gc + 4, :], in_=ohp)

            _mark(nc, "outproj")
            # ---- output projection (partial: this core's two head rows) --
            for gc in range(0, NB, 4):
                of_s = outp.tile([P, 4, DIM], f16, tag="ofs")
                for g4 in range(4):
                    g = gc + g4
                    pf = pp512.tile([P, DIM], f32, tag="p512", name="pf")
                    nc.tensor.matmul(pf, hoT[:, g, :], wout_sb,
                                     start=True, stop=True)
                    nc.scalar.copy(out=of_s[:, g4, :], in_=pf)
                nc.sync.dma_start(
                    out[gc * P:(gc + 4) * P, :].rearrange(
                        "(g p) c -> p g c", p=P), of_s)

    _mark(nc, "tile_finish")
    nc.compile()
    _mark(nc, None)
    return nc


def _prep_mv(mv_slice):
    """[2,2048,32,64] -> [2,16,128,32,65] bf16: partition (ql j) stacks the 4
    stride-32 queries of each group; col 64 = 1.0 (softmax-denominator row)."""
    import ml_dtypes
    r = mv_slice.reshape(2, NB, 4, KNN, KNN, D).transpose(0, 1, 2, 4, 3, 5)
    out = np.empty((2, NB, P, KNN, D + 1), dtype=ml_dtypes.bfloat16)
    out[..., :D] = r.reshape(2, NB, P, KNN, D).astype(np.float16)
    out[..., D] = 1.0
    return out


def _prepare_in_maps(x, w_q, w_kv, w_out, scale_param, mem_k, mem_v, mem_mask,
                     use_mbias):
    f = np.float32
    h = np.float16
    scales8 = np.exp(scale_param.reshape(HEADS).astype(f))
    in_maps = []
    for c in range(NCORES):
        b = c // 4
        h0 = 2 * (c % 4)
        sc = np.empty((P, 4), dtype=f)
        sc[:, 0] = scales8[h0]
        sc[:, 1] = scales8[h0 + 1]
        sc[:, 2] = -scales8[h0]
        sc[:, 3] = -scales8[h0 + 1]
        wqv = np.concatenate(
            [w_q[:, h0 * D:(h0 + 2) * D], w_kv[:, D:2 * D]], axis=1)
        m = {
            "xb": np.ascontiguousarray(x[b], dtype=h),
            "wqv": np.ascontiguousarray(wqv, dtype=h),
            "wk": np.ascontiguousarray(w_kv[:, 0:D], dtype=h),
            "wout2": np.ascontiguousarray(w_out[h0 * D:(h0 + 2) * D, :],
                                          dtype=h),
            "scales": sc,
            "mk": np.ascontiguousarray(
                mem_k[b, h0:h0 + 2].reshape(2, NB, P, KNN, D), dtype=h),
            "mv": _prep_mv(mem_v[b, h0:h0 + 2]),
        }
        if use_mbias:
            mb = np.where(mem_mask[b, h0:h0 + 2], h(0), h(-30000)).astype(h)
            m["mbias"] = np.ascontiguousarray(mb.reshape(2, NB, P, KNN))
        in_maps.append(m)
    return in_maps


def _run(x, w_q, w_kv, w_out, scale_param, mem_k, mem_v, mem_mask, trace=False):
    from concourse.bass_utils import run_bass_kernel_spmd

    use_mbias = not bool(np.all(mem_mask))
    nc = _build(use_mbias)
    in_maps = _prepare_in_maps(x, w_q, w_kv, w_out, scale_param,
                               mem_k, mem_v, mem_mask, use_mbias)
    res = run_bass_kernel_spmd(nc, in_maps, core_ids=list(range(NCORES)),
                               trace=trace)
    out = np.zeros((B, N, DIM), dtype=np.float32)
    for c in range(NCORES):
        out[c // 4] += res.results[c]["out"]
    return out, res


def kernel(x, w_q, w_kv, w_out, scale_param, mem_k, mem_v, mem_mask):
    trace = bool(int(os.environ.get("BASS_KERNEL_TRACE", "0")))
    out, _ = _run(x, w_q, w_kv, w_out, scale_param, mem_k, mem_v, mem_mask,
                  trace=trace)
    return out
